# revision 3
# baseline (speedup 1.0000x reference)
"""Multi-head attention (B=2, S=2048, E=1024, H=16, hd=64) on 8 trn2 cores.

Sharding: core c handles batch b = c//4 and 4 heads h0 = 4*(c%4).
Each core computes its heads' attention output projected through its rows
of Wo (tensor-parallel row split); the host sums the 4 bf16 partials per
batch and adds the (bias-folded) bo.

v2 dataflow (per core, feature-major):
  xT   [e,s]   bf16  <- host-pretransposed x[b]
  q/kT [d2,s]  bf16  <- Wqk^T @ xT          (bf16 matmul, plain copy out)
  v1   [s,d]   bf16  <- xT_sblk^T @ Wv'     (direct k-major V, no PE transpose)
  scores[k, h2*512+q] f32, one [128,1024] psum pair-tile per k-tile
  ex   [k,2*512] bf16 <- exp(0.125*sc (+kappa bias)); causal mul on gpsimd
  ops  [65,q]  f32   <- [v|1]^T @ ex        (bf16 matmul)
  oT   [d,q]   bf16  <- ops * gpsimd-broadcast(1/den)
  y    [q,e]   bf16  <- oT^T @ Wo           (bf16 matmul)

Bias handling (exact): v-bias and bo folded host-side into bo_eff;
q-bias enters softmax only via kappa[k] = k·bq/8 (exp per-partition bias,
computed on-device from an extra Wv' column); k-bias contributions are
per-query constants that cancel in softmax.
"""

import os
import sys

sys.path.insert(0, "/opt/trn_rl_repo")

from contextlib import ExitStack

import ml_dtypes
import numpy as np

import concourse.bass as bass
import concourse.tile as tile
from concourse import bacc, mybir
from concourse._compat import with_exitstack
from concourse.bass_utils import run_bass_kernel_spmd

B, S, E, H = 2, 2048, 1024, 16
HD = 64                # head dim
NH = 4                 # heads per core
ET = E // 128          # 8 e-tiles
KT = S // 128          # 16 k tiles
QSB = S // 512         # 4 query super-blocks
VP = 80                # v1 per-head stride (64 v + 1 ones + pad, 32B aligned)
VC = NH * HD + 8       # Wv' cols: 256 v + 4 kappa + 4 pad = 264
F32 = mybir.dt.float32
BF16 = mybir.dt.bfloat16
EXP = mybir.ActivationFunctionType.Exp

_CACHE = {}
LAST_RESULT = None

ALL_PHASES = frozenset({"x", "qkv", "v", "sc", "av", "wo"})


@with_exitstack
def _mha_kernel(ctx: ExitStack, tc: tile.TileContext, x, wqk, wv, wo, cz2, yp,
                kbias=False, phases=ALL_PHASES):
    nc = tc.nc

    const = ctx.enter_context(tc.tile_pool(name="const", bufs=1))
    work = ctx.enter_context(tc.tile_pool(name="work", bufs=1))
    psum = ctx.enter_context(tc.tile_pool(name="psum", bufs=1, space="PSUM"))

    # ---- persistent SBUF tensors ----
    W = const.tile([128, ET, 512], BF16)         # q,k weights (m: qh0 qh1 kh0 kh1)
    Wv = const.tile([128, ET, VC], BF16)         # v weights + kappa cols
    Wo = const.tile([128, 2, E], BF16)           # Wo slice rows, 2 hd-tiles
    cz = const.tile([128, 256], BF16)            # causal 0/1 [k, h2*128+q]
    xT = const.tile([128, ET, S], BF16)          # x[b] transposed, bf16
    qT = const.tile([128, 2, S], BF16)           # [head-pair d, hp, s]
    kT = const.tile([128, 2, S], BF16)
    v1 = const.tile([128, KT, NH * VP], BF16)    # v k-major + ones col per head
    kap = const.tile([128, KT, NH], F32)         # exp bias kappa (q-bias term)

    # ---- constant loads ----
    for t in range(ET):
        eng = nc.sync if t % 2 == 0 else nc.scalar
        eng.dma_start(W[:, t, :], wqk[t * 128:(t + 1) * 128, :].bitcast(BF16))
        eng.dma_start(Wv[:, t, :], wv[t * 128:(t + 1) * 128, :].bitcast(BF16))
    for t in range(2):
        nc.sync.dma_start(Wo[:, t, :], wo[t * 128:(t + 1) * 128, :].bitcast(BF16))
    nc.scalar.dma_start(cz[:], cz2[:, :].bitcast(BF16))
    for h in range(NH):
        nc.vector.memset(v1[:, :, h * VP + HD:h * VP + HD + 1], 1.0)
    if not kbias:
        nc.vector.memset(kap[:], 0.0)

    # ---- x load (host-pretransposed bf16) ----
    for et in range(ET) if "x" in phases else []:
        eng = nc.sync if et % 2 == 0 else nc.scalar
        eng.dma_start(xT[:, et, :], x[et * 128:(et + 1) * 128, :].bitcast(BF16))

    # ablation: placeholder fills for disabled producer phases
    if "x" not in phases:
        nc.vector.memset(xT[:], 0.25)
    if "qkv" not in phases:
        nc.vector.memset(qT[:], 0.25)
        nc.vector.memset(kT[:], 0.25)
    if "v" not in phases:
        nc.vector.memset(v1[:], 0.25)
        for h in range(NH):
            nc.vector.memset(v1[:, :, h * VP + HD:h * VP + HD + 1], 1.0)

    def emit_qk(n):
        ncol = slice(n * 512, (n + 1) * 512)
        for m in range(4):
            ps = psum.tile([128, 512], F32, name="ps", tag="mm", bufs=2)
            for et in range(ET):
                nc.tensor.matmul(
                    ps[:],
                    W[:, et, m * 128:(m + 1) * 128],
                    xT[:, et, ncol],
                    start=(et == 0),
                    stop=(et == ET - 1),
                )
            dst = (qT if m < 2 else kT)[:, m % 2, ncol]
            nc.vector.tensor_copy(dst, ps[:])

    def emit_v(sb):
        ps = psum.tile([128, VC], F32, name="psv", tag="mm", bufs=2)
        for et in range(ET):
            nc.tensor.matmul(
                ps[:],
                xT[:, et, sb * 128:(sb + 1) * 128],
                Wv[:, et, :],
                start=(et == 0),
                stop=(et == ET - 1),
            )
        nc.vector.tensor_copy(
            v1[:, sb, :].rearrange("p (h c) -> p h c", h=NH)[:, :, :HD],
            ps[:, 0:NH * HD].rearrange("p (h c) -> p h c", h=NH),
        )
        if kbias:
            nc.vector.tensor_copy(kap[:, sb, :], ps[:, NH * HD:NH * HD + NH])

    def emit_attention(qsb, hp, oT):
        nkt = (qsb + 1) * 4
        ops = [
            psum.tile([65, 512], F32, name=f"ops{h2}", tag=f"ops{h2}", bufs=1)
            for h2 in range(2)
        ]

        def emit_scores(kt):
            j = kt - qsb * 4
            q0 = max(0, j * 128)
            ex = work.tile([128, 1024], BF16, name="ex", tag="ex", bufs=6)
            if "sc" not in phases:
                nc.vector.memset(ex[:], 0.001)
                return ex
            sc = psum.tile([128, 1024], F32, name="sc", tag="sc", bufs=2)
            for h2 in range(2):
                b0 = 64 * h2
                nc.tensor.matmul(
                    sc[:, h2 * 512 + q0:(h2 + 1) * 512],
                    kT[b0:b0 + 64, hp, kt * 128:(kt + 1) * 128],
                    qT[b0:b0 + 64, hp, qsb * 512 + q0:(qsb + 1) * 512],
                    start=True,
                    stop=True,
                )
            if q0 == 0 and not kbias:
                nc.scalar.activation(ex[:], sc[:], EXP, scale=0.125)
            else:
                for h2 in range(2):
                    bias = kap[:, kt, 2 * hp + h2:2 * hp + h2 + 1] if kbias else 0.0
                    nc.scalar.activation(
                        ex[:, h2 * 512 + q0:(h2 + 1) * 512],
                        sc[:, h2 * 512 + q0:(h2 + 1) * 512],
                        EXP, bias=bias, scale=0.125,
                    )
            if j >= 0:
                e3 = ex.rearrange("p (h q) -> p h q", h=2)[:, :, q0:q0 + 128]
                nc.gpsimd.tensor_mul(
                    e3, e3, cz[:].rearrange("p (h q) -> p h q", h=2)
                )
            return ex

        def emit_av(kt, ex):
            j = kt - qsb * 4
            q0 = max(0, j * 128)
            for h2 in range(2):
                h = 2 * hp + h2
                nc.tensor.matmul(
                    ops[h2][:, q0:],
                    v1[:, kt, h * VP:h * VP + HD + 1],
                    ex[:, h2 * 512 + q0:(h2 + 1) * 512],
                    start=(kt == 0),
                    stop=(kt == nkt - 1),
                )

        if "av" in phases:
            pend = None
            for kt in range(nkt):
                ex = emit_scores(kt)
                if pend is not None:
                    emit_av(*pend)
                pend = (kt, ex)
            emit_av(*pend)
        else:
            for kt in range(nkt):
                emit_scores(kt)
            for h2 in range(2):
                nc.vector.memset(ops[h2][:], 1.0)

        for h2 in range(2):
            rc = work.tile([1, 512], F32, name="rc", tag="rc", bufs=2)
            nc.vector.reciprocal(rc[:], ops[h2][64:65, :])
            rb = work.tile([64, 512], F32, name="rb", tag="rb", bufs=2)
            nc.gpsimd.partition_broadcast(rb[:], rc[:], channels=64)
            nc.vector.tensor_mul(
                oT[64 * h2:64 * h2 + 64, hp, :], ops[h2][0:64, :], rb[:]
            )

    def emit_wo(qsb, oT):
        for qb2 in range(4):
            qb = qsb * 4 + qb2
            yps = [
                psum.tile([128, 512], F32, name=f"yps{ec}", tag="mm", bufs=2)
                for ec in range(2)
            ]
            for hpt in range(2):
                for ec in range(2):
                    nc.tensor.matmul(
                        yps[ec][:],
                        oT[:, hpt, qb2 * 128:(qb2 + 1) * 128],
                        Wo[:, hpt, ec * 512:(ec + 1) * 512],
                        start=(hpt == 0),
                        stop=(hpt == 1),
                    )
            yt = work.tile([128, E], BF16, name="yt", tag="yt", bufs=3)
            for ec in range(2):
                nc.vector.tensor_copy(yt[:, ec * 512:(ec + 1) * 512], yps[ec][:])
            nc.sync.dma_start(yp[qb * 128:(qb + 1) * 128, :], yt[:])

    # ---- interleaved main schedule ----
    for n in range(QSB):
        if "qkv" in phases:
            emit_qk(n)
        if "v" in phases:
            for sb in range(4 * n, 4 * n + 4):
                emit_v(sb)
        oT = work.tile([128, 2, 512], BF16, name="oT", tag="oT", bufs=2)
        if "sc" in phases or "av" in phases:
            for hp in range(2):
                emit_attention(n, hp, oT)
        else:
            nc.vector.memset(oT[:], 0.25)
        if "wo" in phases:
            emit_wo(n, oT)


def _build(repeat=1, loop=0, phases=ALL_PHASES, kbias=False):
    key = ("nc", repeat, loop, tuple(sorted(phases)), kbias)
    if key in _CACHE:
        return _CACHE[key]
    nc = bacc.Bacc("TRN2", target_bir_lowering=False, debug=False, num_devices=8)
    x = nc.dram_tensor("x", [E, S], mybir.dt.uint16, kind="ExternalInput").ap()
    wqk = nc.dram_tensor("wqk", [E, 512], mybir.dt.uint16,
                         kind="ExternalInput").ap()
    wv = nc.dram_tensor("wv", [E, VC], mybir.dt.uint16,
                        kind="ExternalInput").ap()
    wo = nc.dram_tensor("wo", [NH * HD, E], mybir.dt.uint16,
                        kind="ExternalInput").ap()
    cz2 = nc.dram_tensor("cz2", [128, 256], mybir.dt.uint16,
                         kind="ExternalInput").ap()
    yp = nc.dram_tensor("yp", [S, E], BF16, kind="ExternalOutput").ap()
    with tile.TileContext(nc) as tc:
        if loop:
            with tc.For_i(0, loop, 1):
                _mha_kernel(tc, x, wqk, wv, wo, cz2, yp, kbias=kbias,
                            phases=phases)
        else:
            for _ in range(repeat):
                _mha_kernel(tc, x, wqk, wv, wo, cz2, yp, kbias=kbias,
                            phases=phases)
    nc.compile()
    _CACHE[key] = nc
    return nc


def _shard_inputs(x, Wqkv, bqkv, Wo, bo, mask):
    x = np.asarray(x, dtype=np.float32)
    Wqkv = np.asarray(Wqkv, dtype=np.float32)
    bqkv = np.asarray(bqkv, dtype=np.float32)
    Wo = np.asarray(Wo, dtype=np.float32)

    # causal 0/1 [k, q] 128-block, duplicated for the two packed heads
    kk = np.arange(128)
    cz = (kk[:, None] <= kk[None, :]).astype(np.float32)   # k <= q allowed
    cz2 = np.concatenate([cz, cz], axis=1)
    cz2_u16 = cz2.astype(ml_dtypes.bfloat16).view(np.uint16)

    in_maps = []
    for c in range(8):
        b, g = divmod(c, 4)
        h0 = NH * g
        qk_cols = []
        for t in (0, 1):
            for hh in range(NH):
                base = 3 * HD * (h0 + hh) + t * HD
                qk_cols.extend(range(base, base + HD))
        wqk = np.ascontiguousarray(Wqkv[:, qk_cols]).astype(ml_dtypes.bfloat16)
        v_cols = [3 * HD * (h0 + hh) + 2 * HD + d
                  for hh in range(NH) for d in range(HD)]
        wv = np.zeros((E, VC), dtype=np.float32)
        wv[:, :NH * HD] = Wqkv[:, v_cols]
        for hh in range(NH):
            base = 3 * HD * (h0 + hh)
            wk_h = Wqkv[:, base + HD:base + 2 * HD]
            bq_h = bqkv[base:base + HD]
            wv[:, NH * HD + hh] = (wk_h @ bq_h) / 8.0
        wv = wv.astype(ml_dtypes.bfloat16)
        xt = np.ascontiguousarray(x[b].T).astype(ml_dtypes.bfloat16)
        in_maps.append({
            "x": xt.view(np.uint16),
            "wqk": wqk.view(np.uint16),
            "wv": wv.view(np.uint16),
            "wo": np.ascontiguousarray(
                Wo[HD * h0:HD * h0 + NH * HD, :]
            ).astype(ml_dtypes.bfloat16).view(np.uint16),
            "cz2": cz2_u16,
        })
    return in_maps


def kernel(x, Wqkv, bqkv, Wo, bo, mask):
    global LAST_RESULT
    bqkv = np.asarray(bqkv, dtype=np.float32)
    kbias = bool(np.any(bqkv))
    nc = _build(kbias=kbias)
    in_maps = _shard_inputs(x, Wqkv, bqkv, Wo, bo, mask)
    trace = bool(int(os.environ.get("KERNEL_TRACE", "0")))
    res = run_bass_kernel_spmd(nc, in_maps, list(range(8)), trace=trace)
    LAST_RESULT = res

    Wo = np.asarray(Wo, dtype=np.float32)
    bo = np.asarray(bo, dtype=np.float32)
    bv = np.empty((E,), dtype=np.float32)
    for h in range(H):
        bv[HD * h:HD * h + HD] = bqkv[3 * HD * h + 2 * HD:3 * HD * h + 3 * HD]
    bo_eff = bo + bv @ Wo

    y = np.empty((B, S, E), dtype=np.float32)
    for b in range(B):
        acc = res.results[4 * b]["yp"].astype(np.float32)
        for g in range(1, 4):
            acc = acc + res.results[4 * b + g]["yp"].astype(np.float32)
        y[b] = acc + bo_eff[None, :]
    return y


# revision 19
# speedup vs baseline: 1.0335x; 1.0335x over previous
"""Multi-head attention (B=2, S=2048, E=1024, H=16, hd=64) on 8 trn2 cores.

Sharding: core c handles batch b = c//4 and 4 heads h0 = 4*(c%4).
Each core computes its heads' attention output projected through its rows
of Wo (tensor-parallel row split); the host sums the 4 bf16 partials per
batch and adds the (bias-folded) bo.

v2 dataflow (per core, feature-major):
  xT   [e,s]   bf16  <- host-pretransposed x[b]
  q/kT [d2,s]  bf16  <- Wqk^T @ xT          (bf16 matmul, plain copy out)
  v1   [s,d]   bf16  <- xT_sblk^T @ Wv'     (direct k-major V, no PE transpose)
  scores[k, h2*512+q] f32, one [128,1024] psum pair-tile per k-tile
  ex   [k,2*512] bf16 <- exp(0.125*sc (+kappa bias)); causal mul on gpsimd
  ops  [65,q]  f32   <- [v|1]^T @ ex        (bf16 matmul)
  oT   [d,q]   bf16  <- ops * gpsimd-broadcast(1/den)
  y    [q,e]   bf16  <- oT^T @ Wo           (bf16 matmul)

Bias handling (exact): v-bias and bo folded host-side into bo_eff;
q-bias enters softmax only via kappa[k] = k·bq/8 (exp per-partition bias,
computed on-device from an extra Wv' column); k-bias contributions are
per-query constants that cancel in softmax.
"""

import os
import sys

sys.path.insert(0, "/opt/trn_rl_repo")

from collections import deque
from contextlib import ExitStack

import ml_dtypes
import numpy as np

import concourse.bass as bass
import concourse.tile as tile
from concourse import bacc, mybir
from concourse._compat import with_exitstack
from concourse.bass_utils import run_bass_kernel_spmd

B, S, E, H = 2, 2048, 1024, 16
HD = 64                # head dim
NH = 4                 # heads per core
ET = E // 128          # 8 e-tiles
KT = S // 128          # 16 k tiles
QSB = S // 512         # 4 query super-blocks
VP = 80                # v1 per-head stride (64 v + 1 ones + pad, 32B aligned)
VC = NH * HD + 8       # Wv' cols: 256 v + 4 kappa + 4 pad = 264
F32 = mybir.dt.float32
BF16 = mybir.dt.bfloat16
EXP = mybir.ActivationFunctionType.Exp

_CACHE = {}
LAST_RESULT = None

ALL_PHASES = frozenset({"x", "qkv", "v", "sc", "av", "wo"})


@with_exitstack
def _mha_kernel(ctx: ExitStack, tc: tile.TileContext, x, wqk, wv, wo, cz2, yp,
                kbias=False, phases=ALL_PHASES):
    nc = tc.nc

    const = ctx.enter_context(tc.tile_pool(name="const", bufs=1))
    work = ctx.enter_context(tc.tile_pool(name="work", bufs=1))
    psum = ctx.enter_context(tc.tile_pool(name="psum", bufs=1, space="PSUM"))

    # ---- persistent SBUF tensors ----
    W = const.tile([128, ET, 512], BF16)         # q,k weights (m: qh0 qh1 kh0 kh1)
    Wv = const.tile([128, ET, VC], BF16)         # v weights + kappa cols
    Wo = const.tile([128, 2, E], BF16)           # Wo slice rows, 2 hd-tiles
    cz = const.tile([128, 256], BF16)            # causal 0/1 [k, h2*128+q]
    xT = const.tile([128, ET, S], BF16)          # x[b] transposed, bf16
    qT = const.tile([128, 2, S], BF16)           # [head-pair d, hp, s]
    kT = const.tile([128, 2, S], BF16)
    v1 = const.tile([128, KT, NH * VP], BF16)    # v k-major + ones col per head
    kap = const.tile([128, KT, NH], F32)         # exp bias kappa (q-bias term)
    ones64f = const.tile([1, 64], F32, name="ones64f")
    nc.vector.memset(ones64f[:], 1.0)
    ones64 = const.tile([1, 64], mybir.dt.float32r, name="ones64")
    nc.vector.tensor_copy(ones64[:], ones64f[:])

    # ---- constant loads ----
    # weights/constants on the DVE queue; x (and later yp) on the SP queue.
    # First halves of xT plus W land first so the n=0 qk chains start early.
    for t in range(ET):
        nc.sync.dma_start(W[:, t, :], wqk[t * 128:(t + 1) * 128, :].bitcast(BF16))
        if "x" in phases:
            nc.sync.dma_start(xT[:, t, 0:S // 2],
                              x[t * 128:(t + 1) * 128, 0:S // 2].bitcast(BF16))
    for t in range(ET):
        nc.sync.dma_start(Wv[:, t, :], wv[t * 128:(t + 1) * 128, :].bitcast(BF16))
    nc.sync.dma_start(cz[:], cz2[:, :].bitcast(BF16))
    for t in range(2):
        nc.sync.dma_start(Wo[:, t, :], wo[t * 128:(t + 1) * 128, :].bitcast(BF16))
    for et in range(ET) if "x" in phases else []:
        nc.scalar.dma_start(xT[:, et, S // 2:S],
                            x[et * 128:(et + 1) * 128, S // 2:S].bitcast(BF16))
    for h in range(NH):
        nc.vector.memset(v1[:, :, h * VP + HD:h * VP + HD + 1], 1.0)
    if not kbias:
        nc.vector.memset(kap[:], 0.0)

    # ablation: placeholder fills for disabled producer phases
    if "x" not in phases:
        nc.vector.memset(xT[:], 0.25)
    if "qkv" not in phases:
        nc.vector.memset(qT[:], 0.25)
        nc.vector.memset(kT[:], 0.25)
    if "v" not in phases:
        nc.vector.memset(v1[:], 0.25)
        for h in range(NH):
            nc.vector.memset(v1[:, :, h * VP + HD:h * VP + HD + 1], 1.0)

    def chain_qk(n, m):
        ncol = slice(n * 512, (n + 1) * 512)
        ps = psum.tile([128, 512], F32, name="ps", tag="mm", bufs=2)
        for et in range(ET):
            nc.tensor.matmul(
                ps[:],
                W[:, et, m * 128:(m + 1) * 128],
                xT[:, et, ncol],
                start=(et == 0),
                stop=(et == ET - 1),
            )
            if et == 3:
                yield
        dst = (qT if m < 2 else kT)[:, m % 2, ncol]
        nc.vector.tensor_copy(dst, ps[:])

    def chain_v(sb):
        ps = psum.tile([128, VC], F32, name="psv", tag="mm", bufs=2)
        for et in range(ET):
            nc.tensor.matmul(
                ps[:],
                xT[:, et, sb * 128:(sb + 1) * 128],
                Wv[:, et, :],
                start=(et == 0),
                stop=(et == ET - 1),
            )
            if et == 3:
                yield
        nc.vector.tensor_copy(
            v1[:, sb, :].rearrange("p (h c) -> p h c", h=NH)[:, :, :HD],
            ps[:, 0:NH * HD].rearrange("p (h c) -> p h c", h=NH),
        )
        if kbias:
            nc.vector.tensor_copy(kap[:, sb, :], ps[:, NH * HD:NH * HD + NH])

    def drain(queue):
        for g in queue:
            for _ in g:
                pass
        queue.clear()

    def emit_attention(qsb, hp, oT, bg):
        nkt = (qsb + 1) * 4

        def slot():
            while bg:
                try:
                    next(bg[0])
                    return
                except StopIteration:
                    bg.popleft()
        ops = [
            psum.tile([128, 512], F32, name=f"ops{h2}", tag=f"ops{h2}", bufs=1)
            for h2 in range(2)
        ]

        def emit_scores(kt):
            j = kt - qsb * 4
            q0 = max(0, j * 128)
            ex = work.tile([128, 1024], BF16, name="ex", tag="ex", bufs=6)
            if "sc" not in phases:
                nc.vector.memset(ex[:], 0.001)
                return ex
            sc = psum.tile([128, 1024], F32, name="sc", tag="sc", bufs=2)
            for h2 in range(2):
                b0 = 64 * h2
                nc.tensor.matmul(
                    sc[:, h2 * 512 + q0:(h2 + 1) * 512],
                    kT[b0:b0 + 64, hp, kt * 128:(kt + 1) * 128],
                    qT[b0:b0 + 64, hp, qsb * 512 + q0:(qsb + 1) * 512],
                    start=True,
                    stop=True,
                )
            if q0 == 0 and not kbias:
                nc.scalar.activation(ex[:], sc[:], EXP, scale=0.125)
            else:
                for h2 in range(2):
                    bias = kap[:, kt, 2 * hp + h2:2 * hp + h2 + 1] if kbias else 0.0
                    nc.scalar.activation(
                        ex[:, h2 * 512 + q0:(h2 + 1) * 512],
                        sc[:, h2 * 512 + q0:(h2 + 1) * 512],
                        EXP, bias=bias, scale=0.125,
                    )
            if j >= 0:
                e3 = ex.rearrange("p (h q) -> p h q", h=2)[:, :, q0:q0 + 128]
                nc.vector.tensor_mul(
                    e3, e3, cz[:].rearrange("p (h q) -> p h q", h=2)
                )
            return ex

        def emit_av(kt, ex):
            j = kt - qsb * 4
            q0 = max(0, j * 128)
            for h2 in range(2):
                h = 2 * hp + h2
                nc.tensor.matmul(
                    ops[h2][0:HD + 1, q0:],
                    v1[:, kt, h * VP:h * VP + HD + 1],
                    ex[:, h2 * 512 + q0:(h2 + 1) * 512],
                    start=(kt == 0),
                    stop=(kt == nkt - 1),
                )

        if "av" in phases:
            pend = None
            for kt in range(nkt):
                ex = emit_scores(kt)
                slot()
                if pend is not None:
                    emit_av(*pend)
                    slot()
                pend = (kt, ex)
            emit_av(*pend)
            slot()
        else:
            for kt in range(nkt):
                emit_scores(kt)
                slot()
            for h2 in range(2):
                nc.vector.memset(ops[h2][:], 1.0)

        for h2 in range(2):
            rc = work.tile([1, 512], mybir.dt.float32r, name="rc", tag="rc",
                           bufs=2)
            with nc.allow_low_precision(reason="f32r reciprocal feeding matmul"):
                nc.vector.reciprocal(rc[:], ops[h2][64:65, :])
            # broadcast 1/den across 64 partitions via PE
            rb = psum.tile([64, 512], F32, name="rb", tag="mm", bufs=2)
            nc.tensor.matmul(rb[:], ones64[:], rc[:], start=True, stop=True)
            rbs = work.tile([64, 512], F32, name="rbs", tag="rbs", bufs=2)
            nc.vector.tensor_copy(rbs[:], rb[:])
            nc.vector.tensor_mul(
                oT[64 * h2:64 * h2 + 64, hp, :], ops[h2][0:64, :], rbs[:]
            )

    def emit_wo(qsb, oT):
        for qb2 in range(4):
            qb = qsb * 4 + qb2
            yps = [
                psum.tile([128, 512], F32, name=f"yps{ec}", tag="mm", bufs=2)
                for ec in range(2)
            ]
            for hpt in range(2):
                for ec in range(2):
                    nc.tensor.matmul(
                        yps[ec][:],
                        oT[:, hpt, qb2 * 128:(qb2 + 1) * 128],
                        Wo[:, hpt, ec * 512:(ec + 1) * 512],
                        start=(hpt == 0),
                        stop=(hpt == 1),
                    )
            yt = work.tile([128, E], BF16, name="yt", tag="yt", bufs=3)
            for ec in range(2):
                nc.vector.tensor_copy(yt[:, ec * 512:(ec + 1) * 512], yps[ec][:])
            nc.sync.dma_start(yp[qb * 128:(qb + 1) * 128, :], yt[:])

    # ---- interleaved main schedule ----
    for n in range(QSB):
        if n == 0:
            bgq = deque()
            if "qkv" in phases:
                bgq.extend(chain_qk(0, m) for m in range(4))
            if "v" in phases:
                bgq.extend(chain_v(sb) for sb in range(4))
            drain(bgq)
        bgq = deque()
        if n + 1 < QSB:
            if "qkv" in phases:
                bgq.extend(chain_qk(n + 1, m) for m in range(4))
            if "v" in phases:
                bgq.extend(chain_v(sb) for sb in range(4 * n + 4, 4 * n + 8))
        oT = work.tile([128, 2, 512], BF16, name="oT", tag="oT", bufs=2)
        if "sc" in phases or "av" in phases:
            for hp in range(2):
                emit_attention(n, hp, oT, bgq)
        else:
            nc.vector.memset(oT[:], 0.25)
        drain(bgq)
        if "wo" in phases:
            emit_wo(n, oT)


def _build(repeat=1, loop=0, phases=ALL_PHASES, kbias=False):
    key = ("nc", repeat, loop, tuple(sorted(phases)), kbias)
    if key in _CACHE:
        return _CACHE[key]
    nc = bacc.Bacc("TRN2", target_bir_lowering=False, debug=False, num_devices=8)
    x = nc.dram_tensor("x", [E, S], mybir.dt.uint16, kind="ExternalInput").ap()
    wqk = nc.dram_tensor("wqk", [E, 512], mybir.dt.uint16,
                         kind="ExternalInput").ap()
    wv = nc.dram_tensor("wv", [E, VC], mybir.dt.uint16,
                        kind="ExternalInput").ap()
    wo = nc.dram_tensor("wo", [NH * HD, E], mybir.dt.uint16,
                        kind="ExternalInput").ap()
    cz2 = nc.dram_tensor("cz2", [128, 256], mybir.dt.uint16,
                         kind="ExternalInput").ap()
    yp = nc.dram_tensor("yp", [S, E], BF16, kind="ExternalOutput").ap()
    with tile.TileContext(nc) as tc:
        if loop:
            with tc.For_i(0, loop, 1):
                _mha_kernel(tc, x, wqk, wv, wo, cz2, yp, kbias=kbias,
                            phases=phases)
        else:
            for _ in range(repeat):
                _mha_kernel(tc, x, wqk, wv, wo, cz2, yp, kbias=kbias,
                            phases=phases)
    nc.compile()
    _CACHE[key] = nc
    return nc


def _shard_inputs(x, Wqkv, bqkv, Wo, bo, mask):
    x = np.asarray(x, dtype=np.float32)
    Wqkv = np.asarray(Wqkv, dtype=np.float32)
    bqkv = np.asarray(bqkv, dtype=np.float32)
    Wo = np.asarray(Wo, dtype=np.float32)

    # causal 0/1 [k, q] 128-block, duplicated for the two packed heads
    kk = np.arange(128)
    cz = (kk[:, None] <= kk[None, :]).astype(np.float32)   # k <= q allowed
    cz2 = np.concatenate([cz, cz], axis=1)
    cz2_u16 = cz2.astype(ml_dtypes.bfloat16).view(np.uint16)

    in_maps = []
    for c in range(8):
        b, g = divmod(c, 4)
        h0 = NH * g
        qk_cols = []
        for t in (0, 1):
            for hh in range(NH):
                base = 3 * HD * (h0 + hh) + t * HD
                qk_cols.extend(range(base, base + HD))
        wqk = np.ascontiguousarray(Wqkv[:, qk_cols]).astype(ml_dtypes.bfloat16)
        v_cols = [3 * HD * (h0 + hh) + 2 * HD + d
                  for hh in range(NH) for d in range(HD)]
        wv = np.zeros((E, VC), dtype=np.float32)
        wv[:, :NH * HD] = Wqkv[:, v_cols]
        for hh in range(NH):
            base = 3 * HD * (h0 + hh)
            wk_h = Wqkv[:, base + HD:base + 2 * HD]
            bq_h = bqkv[base:base + HD]
            wv[:, NH * HD + hh] = (wk_h @ bq_h) / 8.0
        wv = wv.astype(ml_dtypes.bfloat16)
        xt = np.ascontiguousarray(x[b].T).astype(ml_dtypes.bfloat16)
        in_maps.append({
            "x": xt.view(np.uint16),
            "wqk": wqk.view(np.uint16),
            "wv": wv.view(np.uint16),
            "wo": np.ascontiguousarray(
                Wo[HD * h0:HD * h0 + NH * HD, :]
            ).astype(ml_dtypes.bfloat16).view(np.uint16),
            "cz2": cz2_u16,
        })
    return in_maps


def kernel(x, Wqkv, bqkv, Wo, bo, mask):
    global LAST_RESULT
    bqkv = np.asarray(bqkv, dtype=np.float32)
    kbias = bool(np.any(bqkv))
    nc = _build(kbias=kbias)
    in_maps = _shard_inputs(x, Wqkv, bqkv, Wo, bo, mask)
    trace = bool(int(os.environ.get("KERNEL_TRACE", "0")))
    res = run_bass_kernel_spmd(nc, in_maps, list(range(8)), trace=trace)
    LAST_RESULT = res

    Wo = np.asarray(Wo, dtype=np.float32)
    bo = np.asarray(bo, dtype=np.float32)
    bv = np.empty((E,), dtype=np.float32)
    for h in range(H):
        bv[HD * h:HD * h + HD] = bqkv[3 * HD * h + 2 * HD:3 * HD * h + 3 * HD]
    bo_eff = bo + bv @ Wo

    y = np.empty((B, S, E), dtype=np.float32)
    for b in range(B):
        acc = res.results[4 * b]["yp"].astype(np.float32)
        for g in range(1, 4):
            acc = acc + res.results[4 * b + g]["yp"].astype(np.float32)
        y[b] = acc + bo_eff[None, :]
    return y


# revision 21
# speedup vs baseline: 1.1410x; 1.1040x over previous
"""Multi-head attention (B=2, S=2048, E=1024, H=16, hd=64) on 8 trn2 cores.

Sharding: core c handles batch b = c//4 and 4 heads h0 = 4*(c%4).
Each core computes its heads' attention output projected through its rows
of Wo (tensor-parallel row split); the host sums the 4 bf16 partials per
batch and adds the (bias-folded) bo.

v2 dataflow (per core, feature-major):
  xT   [e,s]   bf16  <- host-pretransposed x[b]
  q/kT [d2,s]  bf16  <- Wqk^T @ xT          (bf16 matmul, plain copy out)
  v1   [s,d]   bf16  <- xT_sblk^T @ Wv'     (direct k-major V, no PE transpose)
  scores[k, h2*512+q] f32, one [128,1024] psum pair-tile per k-tile
  ex   [k,2*512] bf16 <- exp(0.125*sc (+kappa bias)); causal mul on gpsimd
  ops  [65,q]  f32   <- [v|1]^T @ ex        (bf16 matmul)
  oT   [d,q]   bf16  <- ops * gpsimd-broadcast(1/den)
  y    [q,e]   bf16  <- oT^T @ Wo           (bf16 matmul)

Bias handling (exact): v-bias and bo folded host-side into bo_eff;
q-bias enters softmax only via kappa[k] = k·bq/8 (exp per-partition bias,
computed on-device from an extra Wv' column); k-bias contributions are
per-query constants that cancel in softmax.
"""

import os
import sys

sys.path.insert(0, "/opt/trn_rl_repo")

from collections import deque
from contextlib import ExitStack

import ml_dtypes
import numpy as np

import concourse.bass as bass
import concourse.tile as tile
from concourse import bacc, mybir
from concourse._compat import with_exitstack
from concourse.bass_utils import run_bass_kernel_spmd

B, S, E, H = 2, 2048, 1024, 16
HD = 64                # head dim
NH = 4                 # heads per core
ET = E // 128          # 8 e-tiles
KT = S // 128          # 16 k tiles
QSB = S // 512         # 4 query super-blocks
VP = 80                # v1 per-head stride (64 v + 1 ones + pad, 32B aligned)
VC = NH * HD + 8       # Wv' cols: 256 v + 4 kappa + 4 pad = 264
F32 = mybir.dt.float32
BF16 = mybir.dt.bfloat16
EXP = mybir.ActivationFunctionType.Exp

_CACHE = {}
LAST_RESULT = None

ALL_PHASES = frozenset({"x", "qkv", "v", "sc", "av", "wo"})


@with_exitstack
def _mha_kernel(ctx: ExitStack, tc: tile.TileContext, x, wqk, wv, wo, cz2, yp,
                kbias=False, phases=ALL_PHASES):
    nc = tc.nc

    const = ctx.enter_context(tc.tile_pool(name="const", bufs=1))
    work = ctx.enter_context(tc.tile_pool(name="work", bufs=1))
    psum = ctx.enter_context(tc.tile_pool(name="psum", bufs=1, space="PSUM"))

    # ---- persistent SBUF tensors ----
    W = const.tile([128, ET, 512], BF16)         # q,k weights (m: qh0 qh1 kh0 kh1)
    Wv = const.tile([128, ET, VC], BF16)         # v weights + kappa cols
    Wo = const.tile([128, 2, E], BF16)           # Wo slice rows, 2 hd-tiles
    cz = const.tile([128, 256], BF16)            # causal 0/1 [k, h2*128+q]
    xT = const.tile([128, ET, S], BF16)          # x[b] transposed, bf16
    qT = const.tile([128, 2, S], BF16)           # [head-pair d, hp, s]
    kT = const.tile([128, 2, S], BF16)
    v1 = const.tile([128, KT, NH * VP], BF16)    # v k-major + ones col per head
    kap = const.tile([128, KT, NH], F32)         # exp bias kappa (q-bias term)
    ones64f = const.tile([1, 64], F32, name="ones64f")
    nc.vector.memset(ones64f[:], 1.0)
    ones64 = const.tile([1, 64], mybir.dt.float32r, name="ones64")
    nc.vector.tensor_copy(ones64[:], ones64f[:])

    # ---- constant loads ----
    # weights/constants on the DVE queue; x (and later yp) on the SP queue.
    # First halves of xT plus W land first so the n=0 qk chains start early.
    for t in range(ET):
        nc.sync.dma_start(W[:, t, :], wqk[t * 128:(t + 1) * 128, :].bitcast(BF16))
        if "x" in phases:
            nc.sync.dma_start(xT[:, t, 0:S // 2],
                              x[t * 128:(t + 1) * 128, 0:S // 2].bitcast(BF16))
    for t in range(ET):
        nc.sync.dma_start(Wv[:, t, :], wv[t * 128:(t + 1) * 128, :].bitcast(BF16))
    nc.sync.dma_start(cz[:], cz2[:, :].bitcast(BF16))
    for t in range(2):
        nc.sync.dma_start(Wo[:, t, :], wo[t * 128:(t + 1) * 128, :].bitcast(BF16))
    for et in range(ET) if "x" in phases else []:
        nc.scalar.dma_start(xT[:, et, S // 2:S],
                            x[et * 128:(et + 1) * 128, S // 2:S].bitcast(BF16))
    for h in range(NH):
        nc.vector.memset(v1[:, :, h * VP + HD:h * VP + HD + 1], 1.0)
    if not kbias:
        nc.vector.memset(kap[:], 0.0)

    # ablation: placeholder fills for disabled producer phases
    if "x" not in phases:
        nc.vector.memset(xT[:], 0.25)
    if "qkv" not in phases:
        nc.vector.memset(qT[:], 0.25)
        nc.vector.memset(kT[:], 0.25)
    if "v" not in phases:
        nc.vector.memset(v1[:], 0.25)
        for h in range(NH):
            nc.vector.memset(v1[:, :, h * VP + HD:h * VP + HD + 1], 1.0)

    def chain_qk(n, m):
        ncol = slice(n * 512, (n + 1) * 512)
        ps = psum.tile([128, 512], F32, name="ps", tag="mm", bufs=2)
        for et in range(ET):
            nc.tensor.matmul(
                ps[:],
                W[:, et, m * 128:(m + 1) * 128],
                xT[:, et, ncol],
                start=(et == 0),
                stop=(et == ET - 1),
            )
            if et == 3:
                yield
        dst = (qT if m < 2 else kT)[:, m % 2, ncol]
        nc.vector.tensor_copy(dst, ps[:])

    def chain_v(sb):
        ps = psum.tile([128, VC], F32, name="psv", tag="mm", bufs=2)
        for et in range(ET):
            nc.tensor.matmul(
                ps[:],
                xT[:, et, sb * 128:(sb + 1) * 128],
                Wv[:, et, :],
                start=(et == 0),
                stop=(et == ET - 1),
            )
            if et == 3:
                yield
        nc.vector.tensor_copy(
            v1[:, sb, :].rearrange("p (h c) -> p h c", h=NH)[:, :, :HD],
            ps[:, 0:NH * HD].rearrange("p (h c) -> p h c", h=NH),
        )
        if kbias:
            nc.vector.tensor_copy(kap[:, sb, :], ps[:, NH * HD:NH * HD + NH])

    def drain(queue):
        for g in queue:
            for _ in g:
                pass
        queue.clear()

    def emit_attention(qsb, hp, oT, bg):
        nkt = (qsb + 1) * 4

        def slot():
            while bg:
                try:
                    next(bg[0])
                    return
                except StopIteration:
                    bg.popleft()
        ops = [
            psum.tile([128, 512], F32, name=f"ops{h2}", tag=f"ops{h2}", bufs=1)
            for h2 in range(2)
        ]

        def emit_scores(kt):
            j = kt - qsb * 4
            q0 = max(0, j * 128)
            ex = work.tile([128, 1024], BF16, name="ex", tag="ex", bufs=8)
            if "sc" not in phases:
                nc.vector.memset(ex[:], 0.001)
                return ex
            sc = psum.tile([128, 1024], F32, name="sc", tag="sc", bufs=2)
            for h2 in range(2):
                b0 = 64 * h2
                nc.tensor.matmul(
                    sc[:, h2 * 512 + q0:(h2 + 1) * 512],
                    kT[b0:b0 + 64, hp, kt * 128:(kt + 1) * 128],
                    qT[b0:b0 + 64, hp, qsb * 512 + q0:(qsb + 1) * 512],
                    start=True,
                    stop=True,
                )
            if q0 == 0 and not kbias:
                nc.scalar.activation(ex[:], sc[:], EXP, scale=0.125)
            else:
                for h2 in range(2):
                    bias = kap[:, kt, 2 * hp + h2:2 * hp + h2 + 1] if kbias else 0.0
                    nc.scalar.activation(
                        ex[:, h2 * 512 + q0:(h2 + 1) * 512],
                        sc[:, h2 * 512 + q0:(h2 + 1) * 512],
                        EXP, bias=bias, scale=0.125,
                    )
            if j >= 0:
                e3 = ex.rearrange("p (h q) -> p h q", h=2)[:, :, q0:q0 + 128]
                nc.vector.tensor_mul(
                    e3, e3, cz[:].rearrange("p (h q) -> p h q", h=2)
                )
            return ex

        def emit_av(kt, ex):
            j = kt - qsb * 4
            q0 = max(0, j * 128)
            for h2 in range(2):
                h = 2 * hp + h2
                nc.tensor.matmul(
                    ops[h2][0:HD + 1, q0:],
                    v1[:, kt, h * VP:h * VP + HD + 1],
                    ex[:, h2 * 512 + q0:(h2 + 1) * 512],
                    start=(kt == 0),
                    stop=(kt == nkt - 1),
                )

        if "av" in phases:
            DEPTH = 2
            pend = deque()
            for kt in range(nkt):
                ex = emit_scores(kt)
                slot()
                if len(pend) >= DEPTH:
                    emit_av(*pend.popleft())
                    slot()
                pend.append((kt, ex))
            while pend:
                emit_av(*pend.popleft())
                slot()
        else:
            for kt in range(nkt):
                emit_scores(kt)
                slot()
            for h2 in range(2):
                nc.vector.memset(ops[h2][:], 1.0)

        for h2 in range(2):
            rc = work.tile([1, 512], mybir.dt.float32r, name="rc", tag="rc",
                           bufs=2)
            with nc.allow_low_precision(reason="f32r reciprocal feeding matmul"):
                nc.vector.reciprocal(rc[:], ops[h2][64:65, :])
            # broadcast 1/den across 64 partitions via PE
            rb = psum.tile([64, 512], F32, name="rb", tag="mm", bufs=2)
            nc.tensor.matmul(rb[:], ones64[:], rc[:], start=True, stop=True)
            rbs = work.tile([64, 512], F32, name="rbs", tag="rbs", bufs=2)
            nc.vector.tensor_copy(rbs[:], rb[:])
            nc.vector.tensor_mul(
                oT[64 * h2:64 * h2 + 64, hp, :], ops[h2][0:64, :], rbs[:]
            )

    def emit_wo(qsb, oT):
        for qb2 in range(4):
            qb = qsb * 4 + qb2
            yps = [
                psum.tile([128, 512], F32, name=f"yps{ec}", tag="mm", bufs=2)
                for ec in range(2)
            ]
            for hpt in range(2):
                for ec in range(2):
                    nc.tensor.matmul(
                        yps[ec][:],
                        oT[:, hpt, qb2 * 128:(qb2 + 1) * 128],
                        Wo[:, hpt, ec * 512:(ec + 1) * 512],
                        start=(hpt == 0),
                        stop=(hpt == 1),
                    )
            yt = work.tile([128, E], BF16, name="yt", tag="yt", bufs=3)
            for ec in range(2):
                nc.vector.tensor_copy(yt[:, ec * 512:(ec + 1) * 512], yps[ec][:])
            nc.sync.dma_start(yp[qb * 128:(qb + 1) * 128, :], yt[:])

    # ---- interleaved main schedule ----
    for n in range(QSB):
        if n == 0:
            bgq = deque()
            if "qkv" in phases:
                bgq.extend(chain_qk(0, m) for m in range(4))
            if "v" in phases:
                bgq.extend(chain_v(sb) for sb in range(4))
            drain(bgq)
        bgq = deque()
        if n + 1 < QSB:
            if "qkv" in phases:
                bgq.extend(chain_qk(n + 1, m) for m in range(4))
            if "v" in phases:
                bgq.extend(chain_v(sb) for sb in range(4 * n + 4, 4 * n + 8))
        oT = work.tile([128, 2, 512], BF16, name="oT", tag="oT", bufs=2)
        if "sc" in phases or "av" in phases:
            for hp in range(2):
                emit_attention(n, hp, oT, bgq)
        else:
            nc.vector.memset(oT[:], 0.25)
        drain(bgq)
        if "wo" in phases:
            emit_wo(n, oT)


def _build(repeat=1, loop=0, phases=ALL_PHASES, kbias=False):
    key = ("nc", repeat, loop, tuple(sorted(phases)), kbias)
    if key in _CACHE:
        return _CACHE[key]
    nc = bacc.Bacc("TRN2", target_bir_lowering=False, debug=False, num_devices=8)
    x = nc.dram_tensor("x", [E, S], mybir.dt.uint16, kind="ExternalInput").ap()
    wqk = nc.dram_tensor("wqk", [E, 512], mybir.dt.uint16,
                         kind="ExternalInput").ap()
    wv = nc.dram_tensor("wv", [E, VC], mybir.dt.uint16,
                        kind="ExternalInput").ap()
    wo = nc.dram_tensor("wo", [NH * HD, E], mybir.dt.uint16,
                        kind="ExternalInput").ap()
    cz2 = nc.dram_tensor("cz2", [128, 256], mybir.dt.uint16,
                         kind="ExternalInput").ap()
    yp = nc.dram_tensor("yp", [S, E], BF16, kind="ExternalOutput").ap()
    with tile.TileContext(nc) as tc:
        if loop:
            with tc.For_i(0, loop, 1):
                _mha_kernel(tc, x, wqk, wv, wo, cz2, yp, kbias=kbias,
                            phases=phases)
        else:
            for _ in range(repeat):
                _mha_kernel(tc, x, wqk, wv, wo, cz2, yp, kbias=kbias,
                            phases=phases)
    nc.compile()
    _CACHE[key] = nc
    return nc


def _shard_inputs(x, Wqkv, bqkv, Wo, bo, mask):
    x = np.asarray(x, dtype=np.float32)
    Wqkv = np.asarray(Wqkv, dtype=np.float32)
    bqkv = np.asarray(bqkv, dtype=np.float32)
    Wo = np.asarray(Wo, dtype=np.float32)

    # causal 0/1 [k, q] 128-block, duplicated for the two packed heads
    kk = np.arange(128)
    cz = (kk[:, None] <= kk[None, :]).astype(np.float32)   # k <= q allowed
    cz2 = np.concatenate([cz, cz], axis=1)
    cz2_u16 = cz2.astype(ml_dtypes.bfloat16).view(np.uint16)

    in_maps = []
    for c in range(8):
        b, g = divmod(c, 4)
        h0 = NH * g
        qk_cols = []
        for t in (0, 1):
            for hh in range(NH):
                base = 3 * HD * (h0 + hh) + t * HD
                qk_cols.extend(range(base, base + HD))
        wqk = np.ascontiguousarray(Wqkv[:, qk_cols]).astype(ml_dtypes.bfloat16)
        v_cols = [3 * HD * (h0 + hh) + 2 * HD + d
                  for hh in range(NH) for d in range(HD)]
        wv = np.zeros((E, VC), dtype=np.float32)
        wv[:, :NH * HD] = Wqkv[:, v_cols]
        for hh in range(NH):
            base = 3 * HD * (h0 + hh)
            wk_h = Wqkv[:, base + HD:base + 2 * HD]
            bq_h = bqkv[base:base + HD]
            wv[:, NH * HD + hh] = (wk_h @ bq_h) / 8.0
        wv = wv.astype(ml_dtypes.bfloat16)
        xt = np.ascontiguousarray(x[b].T).astype(ml_dtypes.bfloat16)
        in_maps.append({
            "x": xt.view(np.uint16),
            "wqk": wqk.view(np.uint16),
            "wv": wv.view(np.uint16),
            "wo": np.ascontiguousarray(
                Wo[HD * h0:HD * h0 + NH * HD, :]
            ).astype(ml_dtypes.bfloat16).view(np.uint16),
            "cz2": cz2_u16,
        })
    return in_maps


def kernel(x, Wqkv, bqkv, Wo, bo, mask):
    global LAST_RESULT
    bqkv = np.asarray(bqkv, dtype=np.float32)
    kbias = bool(np.any(bqkv))
    nc = _build(kbias=kbias)
    in_maps = _shard_inputs(x, Wqkv, bqkv, Wo, bo, mask)
    trace = bool(int(os.environ.get("KERNEL_TRACE", "0")))
    res = run_bass_kernel_spmd(nc, in_maps, list(range(8)), trace=trace)
    LAST_RESULT = res

    Wo = np.asarray(Wo, dtype=np.float32)
    bo = np.asarray(bo, dtype=np.float32)
    bv = np.empty((E,), dtype=np.float32)
    for h in range(H):
        bv[HD * h:HD * h + HD] = bqkv[3 * HD * h + 2 * HD:3 * HD * h + 3 * HD]
    bo_eff = bo + bv @ Wo

    y = np.empty((B, S, E), dtype=np.float32)
    for b in range(B):
        acc = res.results[4 * b]["yp"].astype(np.float32)
        for g in range(1, 4):
            acc = acc + res.results[4 * b + g]["yp"].astype(np.float32)
        y[b] = acc + bo_eff[None, :]
    return y
